# revision 28
# baseline (speedup 1.0000x reference)
"""DiffAttn kernel for 8 trn2 NeuronCores.

Problem (per reference):
  X [4, 4096, 1024]; Wq/Wk [1024, 256]; Wv [1024, 128]; biases; lam scalar.
  Q,K = X@Wq+bq, X@Wk+bk ; V = X@Wv+bv
  A_i = Q_i @ K_i^T / sqrt(128)  (i = 1,2 : the two 128-wide halves)
  out = (softmax(A1) - lam * softmax(A2)) @ V          -> [4, 4096, 128]

Sharding: 8 cores = 4 batches x 2 query-halves. Each core computes the
attention output for 2048 queries of one batch; K/V projections for the
full 4096 keys of that batch are computed redundantly on both cores of the
pair (no collectives). Host passes X^T per core with the core's query rows
ordered first; key order is irrelevant to softmax as long as K and V agree.

Per-core dataflow (all matmuls bf16: measured ~1.45x faster than f32r on
real TRN2 silicon; rel-err stays ~6e-3 vs the 2e-2 gate):
  The attention runs as two component passes (softmax1 over K1/Q1, then
  softmax2 over K2/Q2), which frees enough PSUM that the projections no
  longer need a serial phase: only chunks 0-1 of K1/Q1/V are projected up
  front, and every remaining projection group (8 matmuls + a DVE/GpSimd
  bias evacuation) is INJECTED into the attention skt loop, paced two
  chunks ahead of the score matmul that consumes it.  The ScalarE does
  nothing but the 128 exp's (its structural floor); bias-adds, the bf16
  softmax-denominator accumulation (2x DVE mode), and the finalize all
  live on DVE/GpSimd.  Scores are computed transposed S^T[sk, sq] per
  1024-query super-chunk; PV keeps the V tile stationary with E^T moving,
  accumulated over 32 key tiles in PSUM; output ships as O^T [128, 2048]
  and the host transposes (pure layout).
"""

import os
import sys

sys.path.insert(0, "/opt/trn_rl_repo")

from collections import deque

import numpy as np

import concourse.bacc as bacc
import concourse.mybir as mybir
from concourse import masks
from concourse.tile import TileContext
from concourse.bass_utils import run_bass_kernel_spmd

F32 = mybir.dt.float32
BF16 = mybir.dt.bfloat16
AF = mybir.ActivationFunctionType

D = 128
EMB = 1024
B, S = 4, 4096
NQ = S // 2          # queries per core
SQC = 512            # projection column chunk
NCC = S // SQC       # 8 projection column chunks
NE = EMB // 128      # 8 emb tiles
SUP = 1024           # attention query super-chunk (2 psum banks)
NSUP = NQ // SUP     # 2
NSK = S // 128       # 32 key tiles
INV_SQRT_D = 1.0 / np.sqrt(np.float32(D))

TRACE = False
TRACE_DIR = None
LAST_RESULT = None


def _build():
    nc = bacc.Bacc("TRN2", target_bir_lowering=False, debug=False, num_devices=8)

    xt = nc.dram_tensor("xt", [EMB, S], BF16, kind="ExternalInput")
    wq = nc.dram_tensor("wq", [EMB, 2 * D], BF16, kind="ExternalInput")
    wk = nc.dram_tensor("wk", [EMB, 2 * D], BF16, kind="ExternalInput")
    wv = nc.dram_tensor("wv", [EMB, D], BF16, kind="ExternalInput")
    bq = nc.dram_tensor("bq", [2 * D, 1], F32, kind="ExternalInput")
    bk = nc.dram_tensor("bk", [2 * D, 1], F32, kind="ExternalInput")
    bv = nc.dram_tensor("bv", [D, 1], F32, kind="ExternalInput")
    lamv = nc.dram_tensor("lamv", [128, 1], F32, kind="ExternalInput")
    out = nc.dram_tensor("o", [D, NQ], F32, kind="ExternalOutput")  # O^T

    from contextlib import ExitStack

    with TileContext(nc) as tc, ExitStack() as ctx:
        xpool = ctx.enter_context(tc.tile_pool(name="xt", bufs=8))

        def load_chunk(cc):
            t = xpool.tile([128, NE, SQC], BF16, tag="xchunk", name=f"xc_{cc}")
            csl = slice(cc * SQC, (cc + 1) * SQC)
            nc.sync.dma_start(
                out=t[:],
                in_=xt[:, csl].rearrange("(t p) s -> p t s", p=128),
            )
            return t

        cpool = ctx.enter_context(tc.tile_pool(name="const", bufs=1))
        ident = cpool.tile([128, 128], BF16)
        masks.make_identity(nc, ident[:])
        ones_f = cpool.tile([128, 1], F32, tag="ones_f")
        nc.vector.memset(ones_f[:], 1.0)
        ones_rf = cpool.tile([1, 128], F32, tag="ones_rf")
        nc.vector.memset(ones_rf[:], 1.0)
        ones_col = cpool.tile([128, 1], BF16, tag="ones_col")
        nc.vector.tensor_copy(ones_col[:], ones_f[:])
        ones_row = cpool.tile([1, 128], BF16, tag="ones_row")
        nc.vector.tensor_copy(ones_row[:], ones_rf[:])

        bq1 = cpool.tile([128, 1], F32, tag="bq1")
        bq2 = cpool.tile([128, 1], F32, tag="bq2")
        bk1 = cpool.tile([128, 1], F32, tag="bk1")
        bk2 = cpool.tile([128, 1], F32, tag="bk2")
        bvt = cpool.tile([128, 1], F32, tag="bvt")
        lam_t = cpool.tile([128, 1], F32, tag="lam")
        nc.gpsimd.dma_start(out=bq1[:], in_=bq[0:128, :])
        nc.gpsimd.dma_start(out=bq2[:], in_=bq[128:256, :])
        nc.gpsimd.dma_start(out=bk1[:], in_=bk[0:128, :])
        nc.gpsimd.dma_start(out=bk2[:], in_=bk[128:256, :])
        nc.gpsimd.dma_start(out=bvt[:], in_=bv[0:128, :])
        nc.gpsimd.dma_start(out=lam_t[:], in_=lamv[:, :])

        wpool = ctx.enter_context(tc.tile_pool(name="w", bufs=1))
        wq1 = wpool.tile([128, NE, 128], BF16, tag="wq1")
        wq2 = wpool.tile([128, NE, 128], BF16, tag="wq2")
        wk1 = wpool.tile([128, NE, 128], BF16, tag="wk1")
        wk2 = wpool.tile([128, NE, 128], BF16, tag="wk2")
        wvt = wpool.tile([128, NE, 128], BF16, tag="wvt")

        def wsrc(w, dsl):
            return w[:, dsl].rearrange("(t p) d -> p t d", p=128)

        # first projection group (k1) streams e-tile-wise: interleave its
        # weight slices with the first XT chunk so the PE starts early;
        # chunk 1 + wq1 follow immediately (scores(0) needs qt1 cols 0:1024),
        # late-pass weights (wk2/wq2) ship last
        xt0 = xpool.tile([128, NE, SQC], BF16, tag="xchunk", name="xc_0")
        xt1 = xpool.tile([128, NE, SQC], BF16, tag="xchunk", name="xc_1")
        for e in range(NE):
            r = slice(e * 128, (e + 1) * 128)
            nc.sync.dma_start(out=wk1[:, e, :], in_=wk[r, 0:128])
            nc.sync.dma_start(out=xt0[:, e, :], in_=xt[r, 0:512])
        nc.gpsimd.dma_start(out=wq1[:], in_=wsrc(wq, slice(0, 128)))
        nc.gpsimd.dma_start(out=wvt[:], in_=wsrc(wv, slice(0, 128)))
        nc.gpsimd.dma_start(out=wk2[:], in_=wsrc(wk, slice(128, 256)))
        nc.gpsimd.dma_start(out=wq2[:], in_=wsrc(wq, slice(128, 256)))
        for e in range(NE):
            r = slice(e * 128, (e + 1) * 128)
            nc.sync.dma_start(out=xt1[:, e, :], in_=xt[r, 512:1024])

        qkv = ctx.enter_context(tc.tile_pool(name="qkv", bufs=1))
        qt1 = qkv.tile([128, NQ], BF16, tag="qt1")
        qt2 = qkv.tile([128, NQ], BF16, tag="qt2")
        kt1 = qkv.tile([128, S], BF16, tag="kt1")
        kt2 = qkv.tile([128, S], BF16, tag="kt2")
        vv = qkv.tile([128, S], BF16, tag="vv")  # col c*128+j = V[key, d]

        epool = ctx.enter_context(tc.tile_pool(name="e", bufs=5))
        pspool = ctx.enter_context(tc.tile_pool(name="paccs", bufs=2))
        fpool = ctx.enter_context(tc.tile_pool(name="fin", bufs=2))
        smpool = ctx.enter_context(tc.tile_pool(name="small", bufs=2))
        vspool = ctx.enter_context(tc.tile_pool(name="vts", bufs=2))

        o1_s = qkv.tile([128, NQ], F32, tag="o1s")

        # psum pools (8 banks):  scores s x2 bufs -> 4, o-accum -> 2,
        # injected projection group -> 1, V transpose -> 1
        spsum = ctx.enter_context(tc.tile_pool(name="spsum", bufs=2, space="PSUM"))
        opsum = ctx.enter_context(tc.tile_pool(name="opsum", bufs=1, space="PSUM"))
        ppsum = ctx.enter_context(tc.tile_pool(name="ppsum", bufs=1, space="PSUM"))
        tpsum = ctx.enter_context(tc.tile_pool(name="tpsum", bufs=1, space="PSUM"))

        chunks = {0: xt0, 1: xt1}

        def get_chunk(cc):
            if cc not in chunks:
                chunks[cc] = load_chunk(cc)
            return chunks[cc]

        # ---------------- projection group emitters ----------------
        def proj_group(dst, w_t, b_t, cc, tag, pool=None, ptag="pj"):
            """project one 512-col chunk for one output group."""
            xt_t = get_chunk(cc)
            csl = slice(cc * SQC, (cc + 1) * SQC)
            pool = pool if pool is not None else ppsum
            ps = pool.tile([128, SQC], F32, tag=ptag, name=f"ps_{tag}_{cc}")
            for t in range(NE):
                nc.tensor.matmul(
                    ps[:], w_t[:, t, :], xt_t[:, t, :],
                    start=(t == 0), stop=(t == NE - 1),
                )
            nc.vector.tensor_scalar_add(dst[:, csl], ps[:], b_t[:, 0:1])

        def proj_v(cc):
            """V chunk: project, bias, transpose into vv (bf16)."""
            xt_t = get_chunk(cc)
            ps = ppsum.tile([128, SQC], F32, tag="pj", name=f"ps_vt_{cc}")
            for t in range(NE):
                nc.tensor.matmul(
                    ps[:], wvt[:, t, :], xt_t[:, t, :],
                    start=(t == 0), stop=(t == NE - 1),
                )
            vt_s = vspool.tile([128, SQC], BF16, tag="vts")
            nc.vector.tensor_scalar_add(vt_s[:], ps[:], bvt[:, 0:1])
            for j in range(SQC // 128):
                tr = tpsum.tile([128, 128], BF16, tag="vtr", name=f"vtr_{cc}_{j}")
                nc.tensor.transpose(
                    tr[:], vt_s[:, j * 128 : (j + 1) * 128], ident[:]
                )
                col = (cc * (SQC // 128) + j) * 128
                nc.vector.tensor_copy(vv[:, col : col + 128], tr[:])

        # ---------------- attention (two component passes) ----------------
        st = {}

        def scores(comp, kt, qt, sup, skt):
            s_ps = spsum.tile([128, SUP], F32, tag="s", name=f"s_{comp}_{sup}_{skt}")
            ksl = slice(skt * 128, (skt + 1) * 128)
            qof = sup * SUP
            for h in range(2):
                nc.tensor.matmul(
                    s_ps[:, h * 512 : (h + 1) * 512],
                    kt[:, ksl],
                    qt[:, qof + h * 512 : qof + (h + 1) * 512],
                    start=True,
                    stop=True,
                )
            st[(comp, sup, skt)] = s_ps

        def attn_pass(comp, sup, kt, qt, jobs, fin_prev=None):
            """one (component, super-chunk) pass with injected proj jobs.
            Returns a deferred finalize closure: the denominator chain of
            pass N runs inside pass N+1 (injected at skt=1) so it never
            blocks the next pass's scores on the PE queue."""
            o_ps = opsum.tile([128, SUP], F32, tag="o", name=f"o_{comp}_{sup}")
            pacc = pspool.tile([128, SUP], BF16, tag="p", name=f"p_{comp}_{sup}")
            e_prev = None

            def exp_of(skt):
                e_t = epool.tile([128, SUP], BF16, tag="e", name=f"e_{comp}_{sup}_{skt}")
                nc.scalar.activation(
                    e_t[:], st.pop((comp, sup, skt))[:], AF.Exp,
                    scale=float(INV_SQRT_D),
                )
                return e_t

            def consume(skt, e_t):
                if skt == 0:
                    nc.vector.tensor_copy(pacc[:], e_t[:])
                elif skt < NSK - 2:
                    nc.vector.tensor_add(pacc[:], pacc[:], e_t[:])
                ksl = slice(skt * 128, (skt + 1) * 128)
                for h in range(2):
                    hsl = slice(h * 512, (h + 1) * 512)
                    nc.tensor.matmul(
                        o_ps[:, hsl], vv[:, ksl], e_t[:, hsl],
                        start=(skt == 0), stop=(skt == NSK - 1),
                    )
                if skt >= NSK - 2:
                    tails.append(e_t)

            tails = []
            scores(comp, kt, qt, sup, 0)
            for skt in range(1, NSK):
                e_prev = exp_of(skt - 1)
                scores(comp, kt, qt, sup, skt)
                if skt == 1 and fin_prev is not None:
                    fin_prev()
                if jobs:
                    jobs.popleft()()
                consume(skt - 1, e_prev)
            e_prev = exp_of(NSK - 1)
            consume(NSK - 1, e_prev)

            # o accumulator evacuates immediately: frees 2 psum banks for
            # the next pass's PV
            f_o = fpool.tile([128, SUP], F32, tag="fo", name=f"fo_{comp}_{sup}")
            nc.vector.tensor_copy(f_o[:], o_ps[:])

            def finalize():
                # denominator per 512-half on the proj psum tag (1 bank):
                # ones-matmul over pacc + the 2 tail e's, reciprocal,
                # broadcast, then apply to the evacuated o
                ib_s = fpool.tile(
                    [128, SUP], F32, tag="ibs", name=f"ibs_{comp}_{sup}"
                )
                r = smpool.tile([1, SUP], F32, tag="r", name=f"r_{comp}_{sup}")
                rr = smpool.tile([1, SUP], BF16, tag="rr", name=f"rr_{comp}_{sup}")
                for h in range(2):
                    hsl = slice(h * 512, (h + 1) * 512)
                    rs = ppsum.tile(
                        [1, 512], F32, tag="pj", name=f"rs_{comp}_{sup}_{h}"
                    )
                    nc.tensor.matmul(
                        rs[0:1, :], ones_col[:], pacc[:, hsl],
                        start=True, stop=False,
                    )
                    for j, e_t in enumerate(tails):
                        nc.tensor.matmul(
                            rs[0:1, :], ones_col[:], e_t[:, hsl],
                            start=False, stop=(j == len(tails) - 1),
                        )
                    nc.vector.reciprocal_approx_fast(
                        out=r[0:1, hsl], in_=rs[0:1, :]
                    )
                    if comp == 1:
                        nc.vector.tensor_scalar_mul(
                            r[0:1, hsl], r[0:1, hsl], lam_t[0:1, 0:1]
                        )
                    nc.vector.tensor_copy(rr[0:1, hsl], r[0:1, hsl])
                    ib = ppsum.tile(
                        [128, 512], F32, tag="pj", name=f"ib_{comp}_{sup}_{h}"
                    )
                    nc.tensor.matmul(
                        ib[:, :], ones_row[:], rr[0:1, hsl],
                        start=True, stop=True,
                    )
                    nc.vector.tensor_copy(ib_s[:, hsl], ib[:, :])
                qsl = slice(sup * SUP, (sup + 1) * SUP)
                if comp == 0:
                    nc.vector.tensor_mul(o1_s[:, qsl], f_o[:], ib_s[:])
                else:
                    f_t = fpool.tile(
                        [128, SUP], F32, tag="f", name=f"f_{comp}_{sup}"
                    )
                    nc.vector.tensor_mul(f_t[:], f_o[:], ib_s[:])
                    nc.vector.tensor_sub(o1_s[:, qsl], o1_s[:, qsl], f_t[:])
                    nc.sync.dma_start(out=out[:, qsl], in_=o1_s[:, qsl])

            return finalize

        # ---------------- schedule ----------------
        # PE pstate warmup: ~4us of dummy matmuls with no DMA dependencies
        # so the tensor engine is at speed when the first projection lands
        warm = ppsum.tile([128, 512], F32, tag="pj", name="warm")
        for i in range(18):
            nc.tensor.matmul(
                warm[:, 0:128], ident[:], ident[:],
                start=(i == 0), stop=(i == 17),
            )

        # P0 minimum: exactly what scores(comp0,sup0,skt=0) reads -- K1 of
        # chunk 0 and Q1 of chunks 0-1 -- on the double-buffered score psum
        proj_group(kt1, wk1, bk1, 0, "k1", pool=spsum, ptag="s")
        proj_group(qt1, wq1, bq1, 0, "q1", pool=spsum, ptag="s")
        proj_group(qt1, wq1, bq1, 1, "q1", pool=spsum, ptag="s")

        def jk1(cc):
            return lambda: proj_group(kt1, wk1, bk1, cc, "k1")

        def jq1(cc):
            return lambda: proj_group(qt1, wq1, bq1, cc, "q1")

        def jk2(cc):
            return lambda: proj_group(kt2, wk2, bk2, cc, "k2")

        def jq2(cc):
            return lambda: proj_group(qt2, wq2, bq2, cc, "q2")

        def jv(cc):
            return lambda: proj_v(cc)

        # pass (0,0): V chunks and remaining K1/Q1, V(c)/K1(c) ahead of the
        # skt=4c score/PV that consumes them (2 jobs per skt)
        jobs00 = deque([
            jv(0), jk1(1), jv(1), jk1(2), jv(2), jk1(3), jv(3), jq1(2),
            jq1(3), jk1(4), jv(4), jk1(5), jv(5), jk1(6), jv(6), jk1(7),
            jv(7),
        ])
        # pass (0,1): K2 chunks 0-3, Q2 chunks 0-1
        jobs01 = deque([jk2(0), jk2(1), jq2(0), jq2(1), jk2(2), jk2(3)])
        # pass (1,0): K2 chunks 4-7 (needed from skt 16), Q2 chunks 2-3
        jobs10 = deque([jk2(4), jk2(5), jq2(2), jq2(3), jk2(6), jk2(7)])

        fin = attn_pass(0, 0, kt1, qt1, jobs00)
        fin = attn_pass(0, 1, kt1, qt1, jobs01, fin_prev=fin)
        fin = attn_pass(1, 0, kt2, qt2, jobs10, fin_prev=fin)
        fin = attn_pass(1, 1, kt2, qt2, deque(), fin_prev=fin)
        fin()

    nc.compile()
    return nc


_NC = None


def _get_nc():
    global _NC
    if _NC is None:
        _NC = _build()
    return _NC


def kernel(X, lam, Wq, bq, Wk, bk, Wv, bv):
    import ml_dtypes

    BFNP = ml_dtypes.bfloat16

    X = np.asarray(X, dtype=np.float32)
    lam_f = float(np.asarray(lam))
    Wq_b = np.ascontiguousarray(np.asarray(Wq, np.float32).astype(BFNP))
    Wk_b = np.ascontiguousarray(np.asarray(Wk, np.float32).astype(BFNP))
    Wv_b = np.ascontiguousarray(np.asarray(Wv, np.float32).astype(BFNP))
    bq_c = np.asarray(bq, np.float32).reshape(2 * D, 1).copy()
    bk_c = np.asarray(bk, np.float32).reshape(2 * D, 1).copy()
    bv_c = np.asarray(bv, np.float32).reshape(D, 1).copy()
    lam_v = np.full((128, 1), lam_f, np.float32)

    nc = _get_nc()

    in_maps = []
    for core in range(8):
        b, h = divmod(core, 2)
        xb = X[b]
        if h == 0:
            xr = xb
        else:
            xr = np.concatenate([xb[NQ:], xb[:NQ]], axis=0)
        xt_a = np.ascontiguousarray(xr.T.astype(BFNP))
        in_maps.append(
            {
                "xt": xt_a,
                "wq": Wq_b,
                "wk": Wk_b,
                "wv": Wv_b,
                "bq": bq_c,
                "bk": bk_c,
                "bv": bv_c,
                "lamv": lam_v,
            }
        )

    global LAST_RESULT
    kwargs = {}
    if TRACE:
        import tempfile

        tdir = tempfile.mkdtemp(dir=TRACE_DIR) if TRACE_DIR else None
        kwargs = dict(trace=True, tmpdir=tdir)
    res = run_bass_kernel_spmd(nc, in_maps, list(range(8)), **kwargs)
    LAST_RESULT = res

    o = np.empty((B, S, D), np.float32)
    for core in range(8):
        b, h = divmod(core, 2)
        o[b, h * NQ : (h + 1) * NQ, :] = res.results[core]["o"].T
    return o


# revision 33
# speedup vs baseline: 1.0197x; 1.0197x over previous
"""DiffAttn kernel for 8 trn2 NeuronCores.

Problem (per reference):
  X [4, 4096, 1024]; Wq/Wk [1024, 256]; Wv [1024, 128]; biases; lam scalar.
  Q,K = X@Wq+bq, X@Wk+bk ; V = X@Wv+bv
  A_i = Q_i @ K_i^T / sqrt(128)  (i = 1,2 : the two 128-wide halves)
  out = (softmax(A1) - lam * softmax(A2)) @ V          -> [4, 4096, 128]

Sharding: 8 cores = 4 batches x 2 query-halves. Each core computes the
attention output for 2048 queries of one batch; K/V projections for the
full 4096 keys of that batch are computed redundantly on both cores of the
pair (no collectives). Host passes X^T per core with the core's query rows
ordered first; key order is irrelevant to softmax as long as K and V agree.

Per-core dataflow (all matmuls bf16: measured ~1.45x faster than f32r on
real TRN2 silicon; rel-err stays ~6e-3 vs the 2e-2 gate):
  The attention runs as two component passes (softmax1 over K1/Q1, then
  softmax2 over K2/Q2), which frees enough PSUM that the projections no
  longer need a serial phase: only chunks 0-1 of K1/Q1/V are projected up
  front, and every remaining projection group (8 matmuls + a DVE/GpSimd
  bias evacuation) is INJECTED into the attention skt loop, paced two
  chunks ahead of the score matmul that consumes it.  The ScalarE does
  nothing but the 128 exp's (its structural floor); bias-adds, the bf16
  softmax-denominator accumulation (2x DVE mode), and the finalize all
  live on DVE/GpSimd.  Scores are computed transposed S^T[sk, sq] per
  1024-query super-chunk; PV keeps the V tile stationary with E^T moving,
  accumulated over 32 key tiles in PSUM; output ships as O^T [128, 2048]
  and the host transposes (pure layout).
"""

import os
import sys

sys.path.insert(0, "/opt/trn_rl_repo")

from collections import deque

import numpy as np

import concourse.bacc as bacc
import concourse.mybir as mybir
from concourse import masks
from concourse.tile import TileContext
from concourse.bass_utils import run_bass_kernel_spmd

F32 = mybir.dt.float32
BF16 = mybir.dt.bfloat16
AF = mybir.ActivationFunctionType

D = 128
EMB = 1024
B, S = 4, 4096
NQ = S // 2          # queries per core
SQC = 512            # projection column chunk
NCC = S // SQC       # 8 projection column chunks
NE = EMB // 128      # 8 emb tiles
SUP = 1024           # attention query super-chunk (2 psum banks)
NSUP = NQ // SUP     # 2
NSK = S // 128       # 32 key tiles
INV_SQRT_D = 1.0 / np.sqrt(np.float32(D))

TRACE = False
TRACE_DIR = None
LAST_RESULT = None


def _build():
    nc = bacc.Bacc("TRN2", target_bir_lowering=False, debug=False, num_devices=8)

    xt = nc.dram_tensor("xt", [EMB, S], BF16, kind="ExternalInput")
    wq = nc.dram_tensor("wq", [EMB, 2 * D], BF16, kind="ExternalInput")
    wk = nc.dram_tensor("wk", [EMB, 2 * D], BF16, kind="ExternalInput")
    wv = nc.dram_tensor("wv", [EMB, D], BF16, kind="ExternalInput")
    bq = nc.dram_tensor("bq", [2 * D, 1], F32, kind="ExternalInput")
    bk = nc.dram_tensor("bk", [2 * D, 1], F32, kind="ExternalInput")
    bv = nc.dram_tensor("bv", [D, 1], F32, kind="ExternalInput")
    lamv = nc.dram_tensor("lamv", [128, 1], F32, kind="ExternalInput")
    out = nc.dram_tensor("o", [D, NQ], F32, kind="ExternalOutput")  # O^T

    from contextlib import ExitStack

    with TileContext(nc) as tc, ExitStack() as ctx:
        xpool = ctx.enter_context(tc.tile_pool(name="xt", bufs=8))

        def load_chunk(cc):
            t = xpool.tile([128, NE, SQC], BF16, tag="xchunk", name=f"xc_{cc}")
            csl = slice(cc * SQC, (cc + 1) * SQC)
            nc.sync.dma_start(
                out=t[:],
                in_=xt[:, csl].rearrange("(t p) s -> p t s", p=128),
            )
            return t

        cpool = ctx.enter_context(tc.tile_pool(name="const", bufs=1))
        ident = cpool.tile([128, 128], BF16)
        masks.make_identity(nc, ident[:])
        ones_f = cpool.tile([128, 1], F32, tag="ones_f")
        nc.vector.memset(ones_f[:], 1.0)
        ones_rf = cpool.tile([1, 128], F32, tag="ones_rf")
        nc.vector.memset(ones_rf[:], 1.0)
        ones_col = cpool.tile([128, 1], BF16, tag="ones_col")
        nc.vector.tensor_copy(ones_col[:], ones_f[:])
        ones_row = cpool.tile([1, 128], BF16, tag="ones_row")
        nc.vector.tensor_copy(ones_row[:], ones_rf[:])

        bq1 = cpool.tile([128, 1], F32, tag="bq1")
        bq2 = cpool.tile([128, 1], F32, tag="bq2")
        bk1 = cpool.tile([128, 1], F32, tag="bk1")
        bk2 = cpool.tile([128, 1], F32, tag="bk2")
        bvt = cpool.tile([128, 1], F32, tag="bvt")
        lam_t = cpool.tile([128, 1], F32, tag="lam")
        nc.gpsimd.dma_start(out=bq1[:], in_=bq[0:128, :])
        nc.gpsimd.dma_start(out=bq2[:], in_=bq[128:256, :])
        nc.gpsimd.dma_start(out=bk1[:], in_=bk[0:128, :])
        nc.gpsimd.dma_start(out=bk2[:], in_=bk[128:256, :])
        nc.gpsimd.dma_start(out=bvt[:], in_=bv[0:128, :])
        nc.gpsimd.dma_start(out=lam_t[:], in_=lamv[:, :])

        wpool = ctx.enter_context(tc.tile_pool(name="w", bufs=1))
        wq1 = wpool.tile([128, NE, 128], BF16, tag="wq1")
        wq2 = wpool.tile([128, NE, 128], BF16, tag="wq2")
        wk1 = wpool.tile([128, NE, 128], BF16, tag="wk1")
        wk2 = wpool.tile([128, NE, 128], BF16, tag="wk2")
        wvt = wpool.tile([128, NE, 128], BF16, tag="wvt")

        def wsrc(w, dsl):
            return w[:, dsl].rearrange("(t p) d -> p t d", p=128)

        # first projection group (k1) streams e-tile-wise: interleave its
        # weight slices with the first XT chunk so the PE starts early;
        # chunk 1 + wq1 follow immediately (scores(0) needs qt1 cols 0:1024),
        # late-pass weights (wk2/wq2) ship last
        xt0 = xpool.tile([128, NE, SQC], BF16, tag="xchunk", name="xc_0")
        xt1 = xpool.tile([128, NE, SQC], BF16, tag="xchunk", name="xc_1")
        for e in range(NE):
            r = slice(e * 128, (e + 1) * 128)
            nc.sync.dma_start(out=wk1[:, e, :], in_=wk[r, 0:128])
            nc.sync.dma_start(out=xt0[:, e, :], in_=xt[r, 0:512])
        nc.gpsimd.dma_start(out=wq1[:], in_=wsrc(wq, slice(0, 128)))
        nc.gpsimd.dma_start(out=wvt[:], in_=wsrc(wv, slice(0, 128)))
        nc.gpsimd.dma_start(out=wk2[:], in_=wsrc(wk, slice(128, 256)))
        nc.gpsimd.dma_start(out=wq2[:], in_=wsrc(wq, slice(128, 256)))
        for e in range(NE):
            r = slice(e * 128, (e + 1) * 128)
            nc.sync.dma_start(out=xt1[:, e, :], in_=xt[r, 512:1024])

        qkv = ctx.enter_context(tc.tile_pool(name="qkv", bufs=1))
        qt1 = qkv.tile([128, NQ], BF16, tag="qt1")
        qt2 = qkv.tile([128, NQ], BF16, tag="qt2")
        kt1 = qkv.tile([128, S], BF16, tag="kt1")
        kt2 = qkv.tile([128, S], BF16, tag="kt2")
        vv = qkv.tile([128, S], BF16, tag="vv")  # col c*128+j = V[key, d]

        epool = ctx.enter_context(tc.tile_pool(name="e", bufs=5))
        pspool = ctx.enter_context(tc.tile_pool(name="paccs", bufs=2))
        fpool = ctx.enter_context(tc.tile_pool(name="fin", bufs=2))
        smpool = ctx.enter_context(tc.tile_pool(name="small", bufs=2))
        vspool = ctx.enter_context(tc.tile_pool(name="vts", bufs=2))

        o1_s = qkv.tile([128, NQ], F32, tag="o1s")

        # psum pools (8 banks):  scores s x2 bufs -> 4, o-accum -> 2,
        # injected projection group -> 1, V transpose -> 1
        spsum = ctx.enter_context(tc.tile_pool(name="spsum", bufs=2, space="PSUM"))
        opsum = ctx.enter_context(tc.tile_pool(name="opsum", bufs=1, space="PSUM"))
        ppsum = ctx.enter_context(tc.tile_pool(name="ppsum", bufs=1, space="PSUM"))
        tpsum = ctx.enter_context(tc.tile_pool(name="tpsum", bufs=1, space="PSUM"))

        chunks = {0: xt0, 1: xt1}

        def get_chunk(cc):
            if cc not in chunks:
                chunks[cc] = load_chunk(cc)
            return chunks[cc]

        # ---------------- projection group emitters ----------------
        warm_state = {"tile": None, "n": 0}

        def warm_mm():
            """dependency-free matmuls that keep the PE busy (and its
            pstate ramping) while the front-end projections wait on DMA."""
            if warm_state["tile"] is None:
                warm_state["tile"] = tpsum.tile(
                    [128, 128], F32, tag="vtr", name="warm"
                )
            w = warm_state["tile"]
            i = warm_state["n"]
            warm_state["n"] = i + 1
            nc.tensor.matmul(w[:], ident[:], ident[:], start=True, stop=True)

        def proj_group(dst, w_t, b_t, cc, tag, pool=None, ptag="pj", warm=False):
            """project one 512-col chunk for one output group."""
            xt_t = get_chunk(cc)
            csl = slice(cc * SQC, (cc + 1) * SQC)
            pool = pool if pool is not None else ppsum
            ps = pool.tile([128, SQC], F32, tag=ptag, name=f"ps_{tag}_{cc}")
            for t in range(NE):
                nc.tensor.matmul(
                    ps[:], w_t[:, t, :], xt_t[:, t, :],
                    start=(t == 0), stop=(t == NE - 1),
                )
                if warm:
                    warm_mm()
                    warm_mm()
            nc.vector.tensor_scalar_add(dst[:, csl], ps[:], b_t[:, 0:1])

        def proj_v(cc):
            """V chunk: project, bias, transpose into vv (bf16)."""
            xt_t = get_chunk(cc)
            ps = ppsum.tile([128, SQC], F32, tag="pj", name=f"ps_vt_{cc}")
            for t in range(NE):
                nc.tensor.matmul(
                    ps[:], wvt[:, t, :], xt_t[:, t, :],
                    start=(t == 0), stop=(t == NE - 1),
                )
            vt_s = vspool.tile([128, SQC], BF16, tag="vts")
            nc.vector.tensor_scalar_add(vt_s[:], ps[:], bvt[:, 0:1])
            for j in range(SQC // 128):
                tr = tpsum.tile([128, 128], BF16, tag="vtr", name=f"vtr_{cc}_{j}")
                nc.tensor.transpose(
                    tr[:], vt_s[:, j * 128 : (j + 1) * 128], ident[:]
                )
                col = (cc * (SQC // 128) + j) * 128
                nc.vector.tensor_copy(vv[:, col : col + 128], tr[:])

        # ---------------- attention (two component passes) ----------------
        st = {}

        def scores(comp, kt, qt, sup, skt):
            s_ps = spsum.tile([128, SUP], F32, tag="s", name=f"s_{comp}_{sup}_{skt}")
            ksl = slice(skt * 128, (skt + 1) * 128)
            qof = sup * SUP
            for h in range(2):
                nc.tensor.matmul(
                    s_ps[:, h * 512 : (h + 1) * 512],
                    kt[:, ksl],
                    qt[:, qof + h * 512 : qof + (h + 1) * 512],
                    start=True,
                    stop=True,
                )
            st[(comp, sup, skt)] = s_ps

        def attn_pass(comp, sup, kt, qt, jobs, fin_prev=None):
            """one (component, super-chunk) pass with injected proj jobs.
            Returns a deferred finalize closure: the denominator chain of
            pass N runs inside pass N+1 (injected at skt=1) so it never
            blocks the next pass's scores on the PE queue."""
            o_ps = opsum.tile([128, SUP], F32, tag="o", name=f"o_{comp}_{sup}")
            pacc = pspool.tile([128, SUP], BF16, tag="p", name=f"p_{comp}_{sup}")
            e_prev = None

            def exp_of(skt):
                e_t = epool.tile([128, SUP], BF16, tag="e", name=f"e_{comp}_{sup}_{skt}")
                nc.scalar.activation(
                    e_t[:], st.pop((comp, sup, skt))[:], AF.Exp,
                    scale=float(INV_SQRT_D),
                )
                return e_t

            def consume(skt, e_t):
                if skt == 0:
                    nc.vector.tensor_copy(pacc[:], e_t[:])
                else:
                    nc.vector.tensor_add(pacc[:], pacc[:], e_t[:])
                ksl = slice(skt * 128, (skt + 1) * 128)
                for h in range(2):
                    hsl = slice(h * 512, (h + 1) * 512)
                    nc.tensor.matmul(
                        o_ps[:, hsl], vv[:, ksl], e_t[:, hsl],
                        start=(skt == 0), stop=(skt == NSK - 1),
                    )
            scores(comp, kt, qt, sup, 0)
            for skt in range(1, NSK):
                e_prev = exp_of(skt - 1)
                scores(comp, kt, qt, sup, skt)
                if skt == 1 and fin_prev is not None:
                    fin_prev()
                if jobs:
                    jobs.popleft()()
                consume(skt - 1, e_prev)
            e_prev = exp_of(NSK - 1)
            consume(NSK - 1, e_prev)

            # o accumulator evacuates immediately: frees 2 psum banks for
            # the next pass's PV
            f_o = fpool.tile([128, SUP], F32, tag="fo", name=f"fo_{comp}_{sup}")
            nc.vector.tensor_copy(f_o[:], o_ps[:])

            def finalize():
                # denominator per 512-half on the proj psum tag (1 bank):
                # ones-matmul over pacc + the 2 tail e's, reciprocal,
                # broadcast, then apply to the evacuated o
                ib_s = fpool.tile(
                    [128, SUP], F32, tag="ibs", name=f"ibs_{comp}_{sup}"
                )
                r = smpool.tile([1, SUP], F32, tag="r", name=f"r_{comp}_{sup}")
                rr = smpool.tile([1, SUP], BF16, tag="rr", name=f"rr_{comp}_{sup}")
                for h in range(2):
                    hsl = slice(h * 512, (h + 1) * 512)
                    rs = ppsum.tile(
                        [1, 512], F32, tag="pj", name=f"rs_{comp}_{sup}_{h}"
                    )
                    nc.tensor.matmul(
                        rs[0:1, :], ones_col[:], pacc[:, hsl],
                        start=True, stop=True,
                    )
                    nc.vector.reciprocal_approx_fast(
                        out=r[0:1, hsl], in_=rs[0:1, :]
                    )
                    if comp == 1:
                        nc.vector.tensor_scalar_mul(
                            r[0:1, hsl], r[0:1, hsl], lam_t[0:1, 0:1]
                        )
                    nc.vector.tensor_copy(rr[0:1, hsl], r[0:1, hsl])
                    ib = ppsum.tile(
                        [128, 512], F32, tag="pj", name=f"ib_{comp}_{sup}_{h}"
                    )
                    nc.tensor.matmul(
                        ib[:, :], ones_row[:], rr[0:1, hsl],
                        start=True, stop=True,
                    )
                    nc.vector.tensor_copy(ib_s[:, hsl], ib[:, :])
                qsl = slice(sup * SUP, (sup + 1) * SUP)
                if comp == 0:
                    nc.vector.tensor_mul(o1_s[:, qsl], f_o[:], ib_s[:])
                else:
                    f_t = fpool.tile(
                        [128, SUP], F32, tag="f", name=f"f_{comp}_{sup}"
                    )
                    nc.vector.tensor_mul(f_t[:], f_o[:], ib_s[:])
                    nc.vector.tensor_sub(o1_s[:, qsl], o1_s[:, qsl], f_t[:])
                    nc.sync.dma_start(out=out[:, qsl], in_=o1_s[:, qsl])

            return finalize

        # ---------------- schedule ----------------
        # P0 minimum: exactly what scores(comp0,sup0,skt=0) reads -- K1 of
        # chunk 0 and Q1 of chunks 0-1 -- on the double-buffered score psum.
        # warm=True fills DMA-wait gaps with dummy matmuls so the PE pstate
        # ramps during the front-end.
        proj_group(kt1, wk1, bk1, 0, "k1", pool=spsum, ptag="s", warm=True)
        proj_group(qt1, wq1, bq1, 0, "q1", pool=spsum, ptag="s", warm=True)
        proj_group(qt1, wq1, bq1, 1, "q1", pool=spsum, ptag="s", warm=True)

        def jk1(cc):
            return lambda: proj_group(kt1, wk1, bk1, cc, "k1")

        def jq1(cc):
            return lambda: proj_group(qt1, wq1, bq1, cc, "q1")

        def jk2(cc):
            return lambda: proj_group(kt2, wk2, bk2, cc, "k2")

        def jq2(cc):
            return lambda: proj_group(qt2, wq2, bq2, cc, "q2")

        def jv(cc):
            return lambda: proj_v(cc)

        # pass (0,0): V chunks and remaining K1/Q1, V(c)/K1(c) ahead of the
        # skt=4c score/PV that consumes them (2 jobs per skt)
        jobs00 = deque([
            jv(0), jk1(1), jv(1), jk1(2), jv(2), jk1(3), jv(3), jq1(2),
            jq1(3), jk1(4), jv(4), jk1(5), jv(5), jk1(6), jv(6), jk1(7),
            jv(7),
        ])
        # pass (0,1): K2 chunks 0-3, Q2 chunks 0-1
        jobs01 = deque([jk2(0), jk2(1), jq2(0), jq2(1), jk2(2), jk2(3)])
        # pass (1,0): K2 chunks 4-7 (needed from skt 16), Q2 chunks 2-3
        jobs10 = deque([jk2(4), jk2(5), jq2(2), jq2(3), jk2(6), jk2(7)])

        fin = attn_pass(0, 0, kt1, qt1, jobs00)
        fin = attn_pass(0, 1, kt1, qt1, jobs01, fin_prev=fin)
        fin = attn_pass(1, 0, kt2, qt2, jobs10, fin_prev=fin)
        fin = attn_pass(1, 1, kt2, qt2, deque(), fin_prev=fin)
        fin()

    nc.compile()
    return nc


_NC = None


def _get_nc():
    global _NC
    if _NC is None:
        _NC = _build()
    return _NC


def kernel(X, lam, Wq, bq, Wk, bk, Wv, bv):
    import ml_dtypes

    BFNP = ml_dtypes.bfloat16

    X = np.asarray(X, dtype=np.float32)
    lam_f = float(np.asarray(lam))
    Wq_b = np.ascontiguousarray(np.asarray(Wq, np.float32).astype(BFNP))
    Wk_b = np.ascontiguousarray(np.asarray(Wk, np.float32).astype(BFNP))
    Wv_b = np.ascontiguousarray(np.asarray(Wv, np.float32).astype(BFNP))
    bq_c = np.asarray(bq, np.float32).reshape(2 * D, 1).copy()
    bk_c = np.asarray(bk, np.float32).reshape(2 * D, 1).copy()
    bv_c = np.asarray(bv, np.float32).reshape(D, 1).copy()
    lam_v = np.full((128, 1), lam_f, np.float32)

    nc = _get_nc()

    in_maps = []
    for core in range(8):
        b, h = divmod(core, 2)
        xb = X[b]
        if h == 0:
            xr = xb
        else:
            xr = np.concatenate([xb[NQ:], xb[:NQ]], axis=0)
        xt_a = np.ascontiguousarray(xr.T.astype(BFNP))
        in_maps.append(
            {
                "xt": xt_a,
                "wq": Wq_b,
                "wk": Wk_b,
                "wv": Wv_b,
                "bq": bq_c,
                "bk": bk_c,
                "bv": bv_c,
                "lamv": lam_v,
            }
        )

    global LAST_RESULT
    kwargs = {}
    if TRACE:
        import tempfile

        tdir = tempfile.mkdtemp(dir=TRACE_DIR) if TRACE_DIR else None
        kwargs = dict(trace=True, tmpdir=tdir)
    res = run_bass_kernel_spmd(nc, in_maps, list(range(8)), **kwargs)
    LAST_RESULT = res

    o = np.empty((B, S, D), np.float32)
    for core in range(8):
        b, h = divmod(core, 2)
        o[b, h * NQ : (h + 1) * NQ, :] = res.results[core]["o"].T
    return o
